# revision 10
# baseline (speedup 1.0000x reference)
"""Trainium2 Bass kernel for SAGAN-style spatial self-attention.

Reference computation (per batch element b):
    xf = x[b].reshape(C, N)                 # C=256, N=H*W=4096
    q = Wq @ xf + bq                        # [32, N]
    k = Wk @ xf + bk                        # [32, N]
    v = Wv @ xf + bv                        # [256, N]
    e[i, j] = q[:, i] . k[:, j]             # [N, N]
    a = softmax_j(e)
    out[:, i] = sum_j v[:, j] a[i, j]
    y = gamma * out + x[b]

Sharding: 8 cores = 4 batches x 2 query-row halves (communication-free).
Each core computes y^T for its 2048 query rows.

Device algorithm (per core), designed around TensorE's lhsT.T @ rhs form:
  - q, k computed in [d, n] layout directly: lhsT = W^T (host-transposed).
  - v^T computed directly as xf.T @ (gamma * Wv^T): lhsT = xf chunks, which
    avoids any device transpose. An extra ones-column on v^T makes the same
    accumulation produce the softmax denominator for free.
  - energy computed TRANSPOSED (e^T[j, i]): lhsT = k chunk, rhs = q. exp via
    ScalarE (no max-subtraction: |e| <= ~40 and exp accumulates in fp32),
    batched over 2 PSUM banks per ACTIVATE to amortize instruction overhead.
  - out^T[i, c] = sum_j S[j, i] (gamma*v^T)[j, c]: lhsT = S tile, rhs = v^T.
    Column 256 of the psum then holds s_i = sum_j exp(e[i, j]).
  - y^T = out_unnorm^T * (1/s_i) + (x^T + gamma*bv)   [host prefolds the bias]
All matmuls run as float32r (full-rate fp32 PE mode); every matmul operand
tensor is declared float32r so its producer rounds on write.

The attention loop is software-pipelined with a one-group skew (energy
matmuls of group t+1 are emitted before the out-accumulation of group t) so
TensorE never waits on ScalarE's exp. Input DMAs are chunked so projections
start while the image is still streaming in.
"""

import sys

import numpy as np

for _p in ("/opt/trn_rl_repo", "/root/.axon_site", "/root/.axon_site/_ro/pypackages"):
    if _p not in sys.path:
        sys.path.insert(0, _p)

B, C, HW, N, D = 4, 256, 64, 4096, 32
NQ = N // 2          # query rows per core
IBLK = 256           # i-block width for the attention pipeline
NIB = NQ // IBLK     # 8 i-blocks per core
NJ = N // 128        # 32 j-chunks
JG = 4               # j-chunks per batched-exp group (2 PSUM banks)
NIC = IBLK // 128    # i-chunks of 128 rows per i-block

_NC_CACHE = {}


def _build_nc():
    import concourse.bass as bass
    import concourse.tile as tile
    from concourse import bacc, mybir

    f32 = mybir.dt.float32
    f32r = mybir.dt.float32r
    Exp = mybir.ActivationFunctionType.Exp
    ts = bass.ts

    nc = bacc.Bacc(None, target_bir_lowering=False, debug=False)

    xf_d = nc.declare_dram_parameter("xf", [C, N], f32, isOutput=False)
    xq_d = nc.declare_dram_parameter("xq", [C, NQ], f32, isOutput=False)
    xbt_d = nc.declare_dram_parameter("xbt", [NQ, C], f32, isOutput=False)
    wqt_d = nc.declare_dram_parameter("wqt", [C, D], f32, isOutput=False)
    wkt_d = nc.declare_dram_parameter("wkt", [C, D], f32, isOutput=False)
    wvt_d = nc.declare_dram_parameter("wvt", [C, C], f32, isOutput=False)
    bq_d = nc.declare_dram_parameter("bq", [D, 1], f32, isOutput=False)
    bk_d = nc.declare_dram_parameter("bk", [D, 1], f32, isOutput=False)
    out_d = nc.declare_dram_parameter("out", [NQ, C], f32, isOutput=True)

    XCH = 4  # xf DMA column chunks (per e-chunk)

    with tile.TileContext(nc) as tc:
        with (
            tc.tile_pool(name="const", bufs=1) as cpool,
            tc.tile_pool(name="sblk", bufs=4) as spool,
            tc.tile_pool(name="ytile", bufs=3) as ypool,
            tc.tile_pool(name="small", bufs=4) as rpool,
        ):
            # ---- prime the ACT exp table while DMAs run ----------------------
            prime_in = rpool.tile([1, 2], f32, tag="prime", name="prime_in")
            prime_out = rpool.tile([1, 2], f32, tag="prime", name="prime_out")
            nc.vector.memset(prime_in[:], 0.0)
            nc.scalar.activation(prime_out[:], prime_in[:], Exp)

            # ---- load inputs (weights and xq first: they gate the pipeline) --
            xf_sb = cpool.tile([128, 2, N], f32r)
            xq_sb = cpool.tile([128, 2, NQ], f32r)
            xbt_sb = cpool.tile([128, NQ // 128, C], f32)
            wq_sb = cpool.tile([128, 2, D], f32r)
            wk_sb = cpool.tile([128, 2, D], f32r)
            wv_sb = cpool.tile([128, 2, C], f32r)
            bq_sb = cpool.tile([D, 1], f32)
            bk_sb = cpool.tile([D, 1], f32)
            for ec in range(2):
                nc.gpsimd.dma_start(wk_sb[:, ec, :], wkt_d[ts(ec, 128), :])
                nc.gpsimd.dma_start(wq_sb[:, ec, :], wqt_d[ts(ec, 128), :])
                nc.gpsimd.dma_start(wv_sb[:, ec, :], wvt_d[ts(ec, 128), :])
            nc.gpsimd.dma_start(bq_sb[:], bq_d[:])
            nc.gpsimd.dma_start(bk_sb[:], bk_d[:])
            for ec in range(2):
                nc.gpsimd.dma_start(xq_sb[:, ec, :], xq_d[ts(ec, 128), :])
            XW = N // XCH
            for cc in range(XCH):
                for ec in range(2):
                    nc.gpsimd.dma_start(
                        xf_sb[:, ec, ts(cc, XW)],
                        xf_d[ts(ec, 128), ts(cc, XW)],
                    )
            # ---- projections -------------------------------------------------
            k_sb = cpool.tile([D, N], f32r)
            q_sb = cpool.tile([D, NQ], f32r)
            vt_sb = cpool.tile([128, NJ, C + 2], f32r)
            ones_sb = cpool.tile([128, NJ, 2], f32)
            nc.vector.memset(ones_sb[:], 1.0)
            nc.vector.tensor_copy(vt_sb[:, :, C : C + 2], ones_sb[:])

            with tc.tile_pool(name="psA", bufs=3, space="PSUM") as psA:
                for qc in range(NQ // 512):
                    ps = psA.tile([D, 512], f32, tag="psA", name=f"psq{qc}")
                    for ec in range(2):
                        nc.tensor.matmul(
                            ps[:],
                            wq_sb[:, ec, :],
                            xq_sb[:, ec, ts(qc, 512)],
                            start=(ec == 0),
                            stop=(ec == 1),
                        )
                    nc.vector.tensor_scalar_add(q_sb[:, ts(qc, 512)], ps[:], bq_sb[:])
                for jc in range(N // 512):
                    ps = psA.tile([D, 512], f32, tag="psA", name=f"psk{jc}")
                    for ec in range(2):
                        nc.tensor.matmul(
                            ps[:],
                            wk_sb[:, ec, :],
                            xf_sb[:, ec, ts(jc, 512)],
                            start=(ec == 0),
                            stop=(ec == 1),
                        )
                    nc.vector.tensor_scalar_add(k_sb[:, ts(jc, 512)], ps[:], bk_sb[:])
                for j in range(NJ):
                    ps = psA.tile([128, C], f32, tag="psA", name=f"psv{j}")
                    for ec in range(2):
                        nc.tensor.matmul(
                            ps[:],
                            xf_sb[:, ec, ts(j, 128)],
                            wv_sb[:, ec, :],
                            start=(ec == 0),
                            stop=(ec == 1),
                        )
                    nc.vector.tensor_copy(vt_sb[:, j, 0:C], ps[:])

            nc.gpsimd.dma_start(
                xbt_sb[:], xbt_d[:].rearrange("(t p) c -> p t c", p=128)
            )

            # ---- attention (software-pipelined, one-group skew) --------------
            groups = [(ib, jg) for ib in range(NIB) for jg in range(NJ // JG)]
            with (
                tc.tile_pool(name="psE", bufs=2, space="PSUM") as psE,
                tc.tile_pool(name="psO", bufs=2 * NIC, space="PSUM") as psO,
            ):
                opss = {}
                s_tiles = {}

                def emit_energy_exp(t):
                    ib, jg = groups[t]
                    if jg == 0:
                        opss[ib] = [
                            psO.tile([128, C + 2], f32, tag="psO", name=f"ops{ib}_{i2}")
                            for i2 in range(NIC)
                        ]
                    eps = psE.tile([128, JG, IBLK], f32, tag="psE", name=f"eps{t}")
                    for g in range(JG):
                        nc.tensor.matmul(
                            eps[:, g, :],
                            k_sb[:, ts(jg * JG + g, 128)],
                            q_sb[:, ts(ib, IBLK)],
                            start=True,
                            stop=True,
                        )
                    s_t = spool.tile([128, JG, IBLK], f32r, tag="sblk", name=f"s{t}")
                    nc.scalar.activation(s_t[:], eps[:], Exp)
                    s_tiles[t] = s_t

                def emit_out(t):
                    ib, jg = groups[t]
                    s_t = s_tiles.pop(t)
                    for ic2 in range(NIC):
                        for g in range(JG):
                            j = jg * JG + g
                            nc.tensor.matmul(
                                opss[ib][ic2][:],
                                s_t[:, g, ts(ic2, 128)],
                                vt_sb[:, j, :],
                                start=(j == 0),
                                stop=(j == NJ - 1),
                            )
                    if jg == NJ // JG - 1:
                        for ic2 in range(NIC):
                            ic = ib * NIC + ic2
                            ops = opss[ib][ic2]
                            r = rpool.tile(
                                [128, 1], f32, tag="small", name=f"r{ib}_{ic2}"
                            )
                            nc.vector.reciprocal(r[:], ops[:, C : C + 1])
                            y = ypool.tile([128, C], f32, tag="ytile", name=f"y{ic}")
                            nc.vector.tensor_scalar_mul(y[:], ops[:, 0:C], r[:])
                            nc.vector.tensor_add(y[:], y[:], xbt_sb[:, ic, :])
                            nc.gpsimd.dma_start(out_d[ts(ic, 128), :], y[:])

                emit_energy_exp(0)
                for t in range(1, len(groups)):
                    emit_energy_exp(t)
                    emit_out(t - 1)
                emit_out(len(groups) - 1)

    nc.compile()
    return nc


def _get_nc():
    if "nc" not in _NC_CACHE:
        _NC_CACHE["nc"] = _build_nc()
    return _NC_CACHE["nc"]


def kernel(x, Wq, bq, Wk, bk, Wv, bv, gamma):
    from concourse.bass_utils import run_bass_kernel_spmd

    x = np.asarray(x, dtype=np.float32)
    gamma_v = float(np.asarray(gamma).reshape(-1)[0])
    xf = x.reshape(B, C, N)
    wqt = np.ascontiguousarray(np.asarray(Wq, np.float32).T)
    wkt = np.ascontiguousarray(np.asarray(Wk, np.float32).T)
    wvt = np.ascontiguousarray(gamma_v * np.asarray(Wv, np.float32).T)
    bq_c = np.asarray(bq, np.float32).reshape(D, 1).copy()
    bk_c = np.asarray(bk, np.float32).reshape(D, 1).copy()
    gbv = (gamma_v * np.asarray(bv, np.float32))[None, :]

    in_maps = []
    for core in range(8):
        b, h = divmod(core, 2)
        sl = slice(h * NQ, (h + 1) * NQ)
        xb = np.ascontiguousarray(xf[b])
        in_maps.append(
            {
                "xf": xb,
                "xq": np.ascontiguousarray(xb[:, sl]),
                "xbt": np.ascontiguousarray(xb[:, sl].T + gbv),
                "wqt": wqt,
                "wkt": wkt,
                "wvt": wvt,
                "bq": bq_c,
                "bk": bk_c,
            }
        )

    nc = _get_nc()
    res = run_bass_kernel_spmd(nc, in_maps, core_ids=list(range(8)))
    y = np.empty((B, C, N), np.float32)
    for core in range(8):
        b, h = divmod(core, 2)
        y[b][:, h * NQ : (h + 1) * NQ] = res.results[core]["out"].T
    return y.reshape(B, C, HW, HW)


# revision 12
# speedup vs baseline: 1.8538x; 1.8538x over previous
"""Trainium2 Bass kernel for SAGAN-style spatial self-attention.

Reference computation (per batch element b):
    xf = x[b].reshape(C, N)                 # C=256, N=H*W=4096
    q = Wq @ xf + bq                        # [32, N]
    k = Wk @ xf + bk                        # [32, N]
    v = Wv @ xf + bv                        # [256, N]
    e[i, j] = q[:, i] . k[:, j]             # [N, N]
    a = softmax_j(e)
    out[:, i] = sum_j v[:, j] a[i, j]
    y = gamma * out + x[b]

Sharding: 8 cores = 4 batches x 2 query-row halves (communication-free).
Each core computes y^T for its 2048 query rows.

Device algorithm (per core), designed around TensorE's lhsT.T @ rhs form:
  - q, k computed in [d, n] layout directly: lhsT = W^T (host-transposed).
  - v^T computed directly as xf.T @ (gamma * Wv^T): lhsT = xf chunks, which
    avoids any device transpose. An extra ones-column on v^T makes the same
    accumulation produce the softmax denominator for free.
  - energy computed TRANSPOSED (e^T[j, i]): lhsT = k chunk, rhs = q. exp via
    ScalarE (no max-subtraction: |e| <= ~40 and exp accumulates in fp32),
    batched over 2 PSUM banks per ACTIVATE to amortize instruction overhead.
  - out^T[i, c] = sum_j S[j, i] (gamma*v^T)[j, c]: lhsT = S tile, rhs = v^T.
    Column 256 of the psum then holds s_i = sum_j exp(e[i, j]).
  - y^T = out_unnorm^T * (1/s_i) + (x^T + gamma*bv)   [host prefolds the bias]
All matmuls run in bf16 (1 cycle/row on the PE; f32r measured 2 cycles/row
on hardware). PSUM accumulation and the softmax/epilogue stay fp32.

The attention loop is software-pipelined with a one-group skew (energy
matmuls of group t+1 are emitted before the out-accumulation of group t) so
TensorE never waits on ScalarE's exp. Input DMAs are chunked so projections
start while the image is still streaming in.
"""

import sys

import numpy as np

for _p in ("/opt/trn_rl_repo", "/root/.axon_site", "/root/.axon_site/_ro/pypackages"):
    if _p not in sys.path:
        sys.path.insert(0, _p)

B, C, HW, N, D = 4, 256, 64, 4096, 32
NQ = N // 2          # query rows per core
IBLK = 256           # i-block width for the attention pipeline
NIB = NQ // IBLK     # 8 i-blocks per core
NJ = N // 128        # 32 j-chunks
JG = 4               # j-chunks per batched-exp group (2 PSUM banks)
NIC = IBLK // 128    # i-chunks of 128 rows per i-block

_NC_CACHE = {}


def _build_nc():
    import concourse.bass as bass
    import concourse.tile as tile
    from concourse import bacc, mybir

    f32 = mybir.dt.float32
    bf16 = mybir.dt.bfloat16
    Exp = mybir.ActivationFunctionType.Exp
    ts = bass.ts

    nc = bacc.Bacc(None, target_bir_lowering=False, debug=False)

    xf_d = nc.declare_dram_parameter("xf", [C, N], bf16, isOutput=False)
    xq_d = nc.declare_dram_parameter("xq", [C, NQ], bf16, isOutput=False)
    xbt_d = nc.declare_dram_parameter("xbt", [NQ, C], f32, isOutput=False)
    wqt_d = nc.declare_dram_parameter("wqt", [C, D], bf16, isOutput=False)
    wkt_d = nc.declare_dram_parameter("wkt", [C, D], bf16, isOutput=False)
    wvt_d = nc.declare_dram_parameter("wvt", [C, C], bf16, isOutput=False)
    bq_d = nc.declare_dram_parameter("bq", [D, 1], f32, isOutput=False)
    bk_d = nc.declare_dram_parameter("bk", [D, 1], f32, isOutput=False)
    out_d = nc.declare_dram_parameter("out", [NQ, C], f32, isOutput=True)

    XCH = 4  # xf DMA column chunks (per e-chunk)

    with tile.TileContext(nc) as tc:
        with (
            tc.tile_pool(name="const", bufs=1) as cpool,
            tc.tile_pool(name="sblk", bufs=4) as spool,
            tc.tile_pool(name="ytile", bufs=3) as ypool,
            tc.tile_pool(name="small", bufs=4) as rpool,
        ):
            # ---- prime the ACT exp table while DMAs run ----------------------
            prime_in = rpool.tile([1, 2], f32, tag="prime", name="prime_in")
            prime_out = rpool.tile([1, 2], f32, tag="prime", name="prime_out")
            nc.vector.memset(prime_in[:], 0.0)
            nc.scalar.activation(prime_out[:], prime_in[:], Exp)

            # ---- load inputs (weights and xq first: they gate the pipeline) --
            xf_sb = cpool.tile([128, 2, N], bf16)
            xq_sb = cpool.tile([128, 2, NQ], bf16)
            xbt_sb = cpool.tile([128, NQ // 128, C], f32)
            wq_sb = cpool.tile([128, 2, D], bf16)
            wk_sb = cpool.tile([128, 2, D], bf16)
            wv_sb = cpool.tile([128, 2, C], bf16)
            bq_sb = cpool.tile([D, 1], f32)
            bk_sb = cpool.tile([D, 1], f32)
            for ec in range(2):
                nc.gpsimd.dma_start(wk_sb[:, ec, :], wkt_d[ts(ec, 128), :])
                nc.gpsimd.dma_start(wq_sb[:, ec, :], wqt_d[ts(ec, 128), :])
                nc.gpsimd.dma_start(wv_sb[:, ec, :], wvt_d[ts(ec, 128), :])
            nc.gpsimd.dma_start(bq_sb[:], bq_d[:])
            nc.gpsimd.dma_start(bk_sb[:], bk_d[:])
            for ec in range(2):
                nc.gpsimd.dma_start(xq_sb[:, ec, :], xq_d[ts(ec, 128), :])
            XW = N // XCH
            for cc in range(XCH):
                for ec in range(2):
                    nc.gpsimd.dma_start(
                        xf_sb[:, ec, ts(cc, XW)],
                        xf_d[ts(ec, 128), ts(cc, XW)],
                    )
            # ---- projections -------------------------------------------------
            k_sb = cpool.tile([D, N], bf16)
            q_sb = cpool.tile([D, NQ], bf16)
            vt_sb = cpool.tile([128, NJ, C + 2], bf16)
            ones_sb = cpool.tile([128, NJ, 2], f32)
            nc.vector.memset(ones_sb[:], 1.0)
            nc.vector.tensor_copy(vt_sb[:, :, C : C + 2], ones_sb[:])

            with tc.tile_pool(name="psA", bufs=3, space="PSUM") as psA:
                for qc in range(NQ // 512):
                    ps = psA.tile([D, 512], f32, tag="psA", name=f"psq{qc}")
                    for ec in range(2):
                        nc.tensor.matmul(
                            ps[:],
                            wq_sb[:, ec, :],
                            xq_sb[:, ec, ts(qc, 512)],
                            start=(ec == 0),
                            stop=(ec == 1),
                        )
                    nc.vector.tensor_scalar_add(q_sb[:, ts(qc, 512)], ps[:], bq_sb[:])
                for jc in range(N // 512):
                    ps = psA.tile([D, 512], f32, tag="psA", name=f"psk{jc}")
                    for ec in range(2):
                        nc.tensor.matmul(
                            ps[:],
                            wk_sb[:, ec, :],
                            xf_sb[:, ec, ts(jc, 512)],
                            start=(ec == 0),
                            stop=(ec == 1),
                        )
                    nc.vector.tensor_scalar_add(k_sb[:, ts(jc, 512)], ps[:], bk_sb[:])
                for j in range(NJ):
                    ps = psA.tile([128, C], f32, tag="psA", name=f"psv{j}")
                    for ec in range(2):
                        nc.tensor.matmul(
                            ps[:],
                            xf_sb[:, ec, ts(j, 128)],
                            wv_sb[:, ec, :],
                            start=(ec == 0),
                            stop=(ec == 1),
                        )
                    nc.vector.tensor_copy(vt_sb[:, j, 0:C], ps[:])

            nc.gpsimd.dma_start(
                xbt_sb[:], xbt_d[:].rearrange("(t p) c -> p t c", p=128)
            )

            # ---- attention (software-pipelined, one-group skew) --------------
            groups = [(ib, jg) for ib in range(NIB) for jg in range(NJ // JG)]
            with (
                tc.tile_pool(name="psE", bufs=2, space="PSUM") as psE,
                tc.tile_pool(name="psO", bufs=2 * NIC, space="PSUM") as psO,
            ):
                opss = {}
                s_tiles = {}

                def emit_energy_exp(t):
                    ib, jg = groups[t]
                    if jg == 0:
                        opss[ib] = [
                            psO.tile([128, C + 2], f32, tag="psO", name=f"ops{ib}_{i2}")
                            for i2 in range(NIC)
                        ]
                    eps = psE.tile([128, JG, IBLK], f32, tag="psE", name=f"eps{t}")
                    for g in range(JG):
                        nc.tensor.matmul(
                            eps[:, g, :],
                            k_sb[:, ts(jg * JG + g, 128)],
                            q_sb[:, ts(ib, IBLK)],
                            start=True,
                            stop=True,
                        )
                    s_t = spool.tile([128, JG, IBLK], bf16, tag="sblk", name=f"s{t}")
                    nc.scalar.activation(s_t[:], eps[:], Exp)
                    s_tiles[t] = s_t

                def emit_out(t):
                    ib, jg = groups[t]
                    s_t = s_tiles.pop(t)
                    for ic2 in range(NIC):
                        for g in range(JG):
                            j = jg * JG + g
                            nc.tensor.matmul(
                                opss[ib][ic2][:],
                                s_t[:, g, ts(ic2, 128)],
                                vt_sb[:, j, :],
                                start=(j == 0),
                                stop=(j == NJ - 1),
                            )
                    if jg == NJ // JG - 1:
                        for ic2 in range(NIC):
                            ic = ib * NIC + ic2
                            ops = opss[ib][ic2]
                            r = rpool.tile(
                                [128, 1], f32, tag="small", name=f"r{ib}_{ic2}"
                            )
                            nc.vector.reciprocal(r[:], ops[:, C : C + 1])
                            y = ypool.tile([128, C], f32, tag="ytile", name=f"y{ic}")
                            nc.vector.tensor_scalar_mul(y[:], ops[:, 0:C], r[:])
                            nc.vector.tensor_add(y[:], y[:], xbt_sb[:, ic, :])
                            nc.gpsimd.dma_start(out_d[ts(ic, 128), :], y[:])

                emit_energy_exp(0)
                for t in range(1, len(groups)):
                    emit_energy_exp(t)
                    emit_out(t - 1)
                emit_out(len(groups) - 1)

    nc.compile()
    return nc


def _get_nc():
    if "nc" not in _NC_CACHE:
        _NC_CACHE["nc"] = _build_nc()
    return _NC_CACHE["nc"]


def kernel(x, Wq, bq, Wk, bk, Wv, bv, gamma):
    import ml_dtypes
    from concourse.bass_utils import run_bass_kernel_spmd

    bf = ml_dtypes.bfloat16
    x = np.asarray(x, dtype=np.float32)
    gamma_v = float(np.asarray(gamma).reshape(-1)[0])
    xf = x.reshape(B, C, N)
    wqt = np.ascontiguousarray(np.asarray(Wq, np.float32).T.astype(bf))
    wkt = np.ascontiguousarray(np.asarray(Wk, np.float32).T.astype(bf))
    wvt = np.ascontiguousarray((gamma_v * np.asarray(Wv, np.float32).T).astype(bf))
    bq_c = np.asarray(bq, np.float32).reshape(D, 1).copy()
    bk_c = np.asarray(bk, np.float32).reshape(D, 1).copy()
    gbv = (gamma_v * np.asarray(bv, np.float32))[None, :]

    in_maps = []
    for core in range(8):
        b, h = divmod(core, 2)
        sl = slice(h * NQ, (h + 1) * NQ)
        xb = np.ascontiguousarray(xf[b])
        xb16 = xb.astype(bf)
        in_maps.append(
            {
                "xf": xb16,
                "xq": np.ascontiguousarray(xb16[:, sl]),
                "xbt": np.ascontiguousarray(xb[:, sl].T + gbv),
                "wqt": wqt,
                "wkt": wkt,
                "wvt": wvt,
                "bq": bq_c,
                "bk": bk_c,
            }
        )

    nc = _get_nc()
    res = run_bass_kernel_spmd(nc, in_maps, core_ids=list(range(8)))
    y = np.empty((B, C, N), np.float32)
    for core in range(8):
        b, h = divmod(core, 2)
        y[b][:, h * NQ : (h + 1) * NQ] = res.results[core]["out"].T
    return y.reshape(B, C, HW, HW)
